# revision 5
# baseline (speedup 1.0000x reference)
"""Trainium2 Bass kernel for topk_masking (nn_ASPM_13700945674777).

Computation (per batch row b):
    h = tanh(x[b] @ W1 + b1)            # (T, D)
    s = h @ w2 + b2                     # (T,)
    mask out the T/2 lowest scores, softmax the rest, out = x * w[:, None]

Strategy (8 NeuronCores, data-parallel over batch, 4 batches/core):
    K1: scores for all rows using float32r matmuls (full PE rate, ~1e-4 err)
    host: per-row rank window (128 scores nearest the cut) selection
    K2: exact-fp32 re-computation of the windowed scores
    host: merge -> exact per-row threshold (2048th smallest)
    K3: mask + softmax + out = x * w   (streaming, DMA-bound)

The f32r error (~1e-4 max) is far below the window's value margin (~1e-2),
and the fp32 refine error (~1e-6) is far below the minimum kept/masked score
gap (9.1e-6 for this input distribution), so the produced mask matches the
fp32 reference exactly.
"""

import numpy as np
import concourse.bass as bass
import concourse.mybir as mybir
from concourse import bacc
from concourse.tile import TileContext
from concourse.bass_utils import run_bass_kernel_spmd
from concourse.masks import make_identity

B, T, D = 32, 4096, 1024
NCORES = 8
BPC = B // NCORES          # batches per core
KEEP = T // 2              # scores kept per row
WIN = 128                  # refinement window size per row
HALF = WIN // 2
TCHUNK = 512               # t-tile for the score pass
NTC = T // TCHUNK
NSUB = TCHUNK // 128       # 128-row subtiles per t-tile
DC = D // 128              # contraction chunks
F32 = mybir.dt.float32
F32R = mybir.dt.float32r

# Populated on every kernel() call: list of (name, BassKernelResults)
LAST_RUN = []


def _load_w1_f32(nc, const, dtype):
    """W1 [D, D] -> SBUF [128, DC, D]; slice [:, dc, e0:e1] is lhsT [128d, e]."""
    w1_d = nc.dram_tensor("w1", [D, D], dtype, kind="ExternalInput")
    w1sb = const.tile([128, DC, D], dtype, tag="w1sb")
    nc.sync.dma_start(out=w1sb, in_=w1_d.rearrange("(c p) e -> p c e", p=128))
    return w1sb


def _load_vec(nc, const, name, dtype, tag):
    """[D] vector -> SBUF [128, DC]; column c holds elements c*128..c*128+127."""
    v_d = nc.dram_tensor(name, [D], dtype, kind="ExternalInput")
    vsb = const.tile([128, DC], dtype, tag=tag)
    nc.sync.dma_start(out=vsb, in_=v_d.rearrange("(c p) -> p c", p=128))
    return vsb


def _score_pass(nc, tc, pools, ident, w1sb, b1sb, w2sb, x_ap, s_out_ap,
                n_rows, mm_dtype):
    """Compute s = tanh(x @ W1 + b1) @ w2 for n_rows rows of x.

    x_ap:     DRAM AP [n_rows, D]
    s_out_ap: DRAM AP [1, n_rows]
    Processes in column blocks of TCHUNK rows stacked along the free dim.
    """
    const, xin, xtp, hp, sp, pst, psh, pss = pools

    nchunks = n_rows // TCHUNK
    for i in range(nchunks):
        # ---- load x rows [TCHUNK, D] as 4 subtiles of 128 rows
        xn = xin.tile([128, NSUB, D], F32, tag="xn")
        nc.sync.dma_start(
            out=xn,
            in_=x_ap[i * TCHUNK:(i + 1) * TCHUNK, :].rearrange(
                "(n p) d -> p n d", p=128),
        )
        # ---- transpose to xT [128d, DC, TCHUNK]
        xT = xtp.tile([128, DC, TCHUNK], mm_dtype, tag="xT")
        for dc in range(DC):
            ptr = pst.tile([128, TCHUNK], F32, tag="ptr")
            for n in range(NSUB):
                nc.tensor.matmul(
                    ptr[:, n * 128:(n + 1) * 128],
                    xn[:, n, dc * 128:(dc + 1) * 128],
                    ident,
                    is_transpose=True,
                    start=(n == 0),
                    stop=(n == NSUB - 1),
                )
            nc.any.tensor_copy(xT[:, dc, :], ptr)
        # ---- hT = tanh(W1.T @ xT + b1), then s += w2.T @ hT
        ps_s = pss.tile([1, TCHUNK], F32, tag="ps_s")
        for ec in range(DC):
            ph = psh.tile([128, TCHUNK], F32, tag="ph")
            for dc in range(DC):
                nc.tensor.matmul(
                    ph,
                    w1sb[:, dc, ec * 128:(ec + 1) * 128],
                    xT[:, dc, :],
                    start=(dc == 0),
                    stop=(dc == DC - 1),
                )
            hT = hp.tile([128, TCHUNK], mm_dtype, tag="hT")
            nc.scalar.activation(
                hT, ph, mybir.ActivationFunctionType.Tanh,
                bias=b1sb[:, ec:ec + 1],
            )
            nc.tensor.matmul(
                ps_s, w2sb[:, ec:ec + 1], hT,
                start=(ec == 0), stop=(ec == DC - 1),
            )
        s_sb = sp.tile([1, TCHUNK], F32, tag="s_sb")
        nc.vector.tensor_copy(s_sb, ps_s)
        nc.sync.dma_start(
            out=s_out_ap[:, i * TCHUNK:(i + 1) * TCHUNK], in_=s_sb)


def _mk_pools(tc, ctx_list, psh_bufs=4):
    mk = tc.tile_pool
    pools = (
        ctx_list.enter_context(mk(name="const", bufs=1)),
        ctx_list.enter_context(mk(name="xin", bufs=2)),
        ctx_list.enter_context(mk(name="xtp", bufs=2)),
        ctx_list.enter_context(mk(name="hp", bufs=2)),
        ctx_list.enter_context(mk(name="sp", bufs=2)),
        ctx_list.enter_context(mk(name="pst", bufs=2, space="PSUM")),
        ctx_list.enter_context(mk(name="psh", bufs=psh_bufs, space="PSUM")),
        ctx_list.enter_context(mk(name="pss", bufs=2, space="PSUM")),
    )
    return pools


def build_k1():
    """Approximate (f32r) scores for the core's BPC batches."""
    from contextlib import ExitStack
    nc = bacc.Bacc()
    x_d = nc.dram_tensor("x", [BPC, T, D], F32, kind="ExternalInput")
    s_d = nc.dram_tensor("s", [BPC, T], F32, kind="ExternalOutput")
    with TileContext(nc) as tc:
        with ExitStack() as ctx:
            pools = _mk_pools(tc, ctx)
            const = pools[0]
            ident = const.tile([128, 128], F32, tag="ident")
            make_identity(nc, ident)
            w1sb = _load_w1_f32(nc, const, F32R)
            b1sb = _load_vec(nc, const, "b1", F32, "b1sb")
            w2sb = _load_vec(nc, const, "w2", F32R, "w2sb")
            for b in range(BPC):
                _score_pass(
                    nc, tc, pools, ident, w1sb, b1sb, w2sb,
                    x_d[b, :, :], s_d[b:b + 1, :], T, F32R)
    nc.finalize()
    return nc


def build_k2():
    """Exact fp32 scores for the BPC*WIN host-gathered window rows."""
    from contextlib import ExitStack
    nrows = BPC * WIN
    nc = bacc.Bacc()
    xg_d = nc.dram_tensor("xg", [nrows, D], F32, kind="ExternalInput")
    s_d = nc.dram_tensor("sref", [1, nrows], F32, kind="ExternalOutput")
    with TileContext(nc) as tc:
        with ExitStack() as ctx:
            pools = _mk_pools(tc, ctx)
            const = pools[0]
            ident = const.tile([128, 128], F32, tag="ident")
            make_identity(nc, ident)
            w1sb = _load_w1_f32(nc, const, F32)
            b1sb = _load_vec(nc, const, "b1", F32, "b1sb")
            w2sb = _load_vec(nc, const, "w2", F32, "w2sb")
            _score_pass(
                nc, tc, pools, ident, w1sb, b1sb, w2sb,
                xg_d[:, :], s_d, nrows, F32)
    nc.finalize()
    return nc


def build_k3():
    """Mask + softmax + out = x * w, given merged scores and thresholds."""
    from contextlib import ExitStack
    nc = bacc.Bacc()
    x_d = nc.dram_tensor("x", [BPC, T, D], F32, kind="ExternalInput")
    sf_d = nc.dram_tensor("sf", [BPC, T], F32, kind="ExternalInput")
    thr_d = nc.dram_tensor("thr", [BPC, 1], F32, kind="ExternalInput")
    out_d = nc.dram_tensor("out", [BPC, T, D], F32, kind="ExternalOutput")
    w_d = nc.dram_tensor("w", [BPC, T], F32, kind="ExternalOutput")
    NB = T // 128              # 32 t-blocks per batch
    OCH = 4                    # t-blocks per output DMA chunk
    with TileContext(nc) as tc:
        with ExitStack() as ctx:
            const = ctx.enter_context(tc.tile_pool(name="const", bufs=1))
            sm = ctx.enter_context(tc.tile_pool(name="sm", bufs=2))
            xin = ctx.enter_context(tc.tile_pool(name="xin", bufs=3))
            op = ctx.enter_context(tc.tile_pool(name="op", bufs=3))
            psp = ctx.enter_context(tc.tile_pool(name="psp", bufs=2, space="PSUM"))
            ident32 = const.tile([32, 32], F32, tag="id32")
            make_identity(nc, ident32)
            ones32 = const.tile([32, 32], F32, tag="ones32")
            nc.vector.memset(ones32, 1.0)
            for b in range(BPC):
                s_np = sm.tile([32, 128], F32, tag="s_np")
                nc.sync.dma_start(
                    out=s_np, in_=sf_d[b, :].rearrange("(n p) -> n p", p=128))
                thr_sb = sm.tile([32, 1], F32, tag="thr_sb")
                nc.sync.dma_start(
                    out=thr_sb, in_=thr_d[b, :].to_broadcast([32, 1]))
                # mask = s > thr ; em = exp(s) * mask
                mask = sm.tile([32, 128], F32, tag="mask")
                nc.vector.tensor_scalar(
                    mask, s_np, thr_sb, None, op0=mybir.AluOpType.is_gt)
                ex = sm.tile([32, 128], F32, tag="ex")
                nc.scalar.activation(ex, s_np, mybir.ActivationFunctionType.Exp)
                em = sm.tile([32, 128], F32, tag="em")
                nc.vector.tensor_tensor(
                    out=em, in0=ex, in1=mask, op=mybir.AluOpType.mult)
                # total = sum(em) replicated across partitions, winv = 1/total
                part = sm.tile([32, 1], F32, tag="part")
                nc.vector.tensor_reduce(
                    part, em, axis=mybir.AxisListType.X, op=mybir.AluOpType.add)
                ps_tot = psp.tile([32, 1], F32, tag="ps_tot")
                nc.tensor.matmul(ps_tot, ones32, part, start=True, stop=True)
                tot = sm.tile([32, 1], F32, tag="tot")
                nc.vector.tensor_copy(tot, ps_tot)
                winv = sm.tile([32, 1], F32, tag="winv")
                nc.vector.reciprocal(winv, tot)
                # weights (row layout) -> DRAM ; transpose for the multiply
                w_np = sm.tile([32, 128], F32, tag="w_np")
                nc.vector.tensor_scalar_mul(w_np, em, winv)
                nc.sync.dma_start(
                    out=w_d[b, :].rearrange("(n p) -> n p", p=128), in_=w_np)
                ps_wt = psp.tile([128, 32], F32, tag="ps_wt")
                nc.tensor.transpose(ps_wt, w_np, ident32)
                w_str = sm.tile([128, 32], F32, tag="w_str")
                nc.vector.tensor_copy(w_str, ps_wt)
                # out = x * w
                for c in range(NB // OCH):
                    xt = xin.tile([128, OCH, D], F32, tag="xt")
                    nc.sync.dma_start(
                        out=xt,
                        in_=x_d[b, c * OCH * 128:(c + 1) * OCH * 128, :]
                        .rearrange("(n p) d -> p n d", p=128),
                    )
                    ot = op.tile([128, OCH, D], F32, tag="ot")
                    for m in range(OCH):
                        nc.vector.tensor_scalar_mul(
                            ot[:, m, :], xt[:, m, :],
                            w_str[:, c * OCH + m:c * OCH + m + 1])
                    nc.sync.dma_start(
                        out=out_d[b, c * OCH * 128:(c + 1) * OCH * 128, :]
                        .rearrange("(n p) d -> p n d", p=128),
                        in_=ot,
                    )
    nc.finalize()
    return nc


_PROGS = {}


def _get_prog(name):
    if name not in _PROGS:
        _PROGS[name] = {"k1": build_k1, "k2": build_k2, "k3": build_k3}[name]()
    return _PROGS[name]


def _run(name, in_maps):
    res = run_bass_kernel_spmd(_get_prog(name), in_maps, core_ids=list(range(NCORES)))
    LAST_RUN.append((name, res))
    return res


def kernel(x, W1, b1, w2, b2):
    LAST_RUN.clear()
    x = np.ascontiguousarray(np.asarray(x, dtype=np.float32))
    W1 = np.ascontiguousarray(np.asarray(W1, dtype=np.float32))
    b1 = np.ascontiguousarray(np.asarray(b1, dtype=np.float32)).reshape(D)
    w2v = np.ascontiguousarray(np.asarray(w2, dtype=np.float32)).reshape(D)
    b2s = float(np.asarray(b2).reshape(-1)[0])

    # ---- K1: approximate scores
    in1 = [
        {"x": x[c * BPC:(c + 1) * BPC], "w1": W1, "b1": b1, "w2": w2v}
        for c in range(NCORES)
    ]
    r1 = _run("k1", in1)
    s1 = np.concatenate([r1.results[c]["s"] for c in range(NCORES)], axis=0)
    s1 = s1 + np.float32(b2s)                       # [B, T]

    # ---- host: rank-window selection
    order = np.argsort(s1, axis=1, kind="stable")
    win_idx = order[:, KEEP - HALF:KEEP + HALF]     # [B, WIN]
    xg = np.ascontiguousarray(
        np.take_along_axis(x, win_idx[:, :, None], axis=1))  # [B, WIN, D]

    # ---- K2: exact fp32 scores for the window rows
    in2 = [
        {"xg": xg[c * BPC:(c + 1) * BPC].reshape(BPC * WIN, D),
         "w1": W1, "b1": b1, "w2": w2v}
        for c in range(NCORES)
    ]
    r2 = _run("k2", in2)
    sref = np.concatenate(
        [r2.results[c]["sref"].reshape(BPC, WIN) for c in range(NCORES)], axis=0)
    sref = sref + np.float32(b2s)                   # [B, WIN]

    # ---- host: merge + exact per-row threshold
    s_final = s1.copy()
    np.put_along_axis(s_final, win_idx, sref, axis=1)
    thr = np.partition(s_final, KEEP - 1, axis=1)[:, KEEP - 1]  # [B]
    counts = (s_final > thr[:, None]).sum(axis=1)
    assert np.all(counts == KEEP), f"threshold tie rows: {np.where(counts != KEEP)}"

    # ---- K3: mask + softmax + multiply
    in3 = [
        {"x": x[c * BPC:(c + 1) * BPC],
         "sf": s_final[c * BPC:(c + 1) * BPC],
         "thr": thr[c * BPC:(c + 1) * BPC].reshape(BPC, 1).astype(np.float32)}
        for c in range(NCORES)
    ]
    r3 = _run("k3", in3)
    out = np.concatenate([r3.results[c]["out"] for c in range(NCORES)], axis=0)
    w_out = np.concatenate([r3.results[c]["w"] for c in range(NCORES)], axis=0)
    return out, w_out


# revision 7
# speedup vs baseline: 24.4332x; 24.4332x over previous
"""Trainium2 Bass kernel for topk_masking (nn_ASPM_13700945674777).

Computation (per batch row b):
    h = tanh(x[b] @ W1 + b1)            # (T, D)
    s = h @ w2 + b2                     # (T,)
    mask out the T/2 lowest scores, softmax the rest, out = x * w[:, None]

Strategy (8 NeuronCores, data-parallel over batch, 4 batches/core):
    K1: scores for all rows using float32r matmuls (full PE rate, ~1e-4 err)
    host: per-row rank window (128 scores nearest the cut) selection
    K2: exact-fp32 re-computation of the windowed scores
    host: merge -> exact per-row threshold (2048th smallest)
    K3: mask + softmax + out = x * w   (streaming, DMA-bound)

The f32r error (~1e-4 max) is far below the window's value margin (~1e-2),
and the fp32 refine error (~1e-6) is far below the minimum kept/masked score
gap (9.1e-6 for this input distribution), so the produced mask matches the
fp32 reference exactly.
"""

import numpy as np
import concourse.bass as bass
import concourse.mybir as mybir
from concourse import bacc
from concourse.tile import TileContext
from concourse.bass_utils import run_bass_kernel_spmd
from concourse.masks import make_identity

B, T, D = 32, 4096, 1024
NCORES = 8
BPC = B // NCORES          # batches per core
KEEP = T // 2              # scores kept per row
WIN = 128                  # refinement window size per row
HALF = WIN // 2
TCHUNK = 512               # t-tile for the score pass
NTC = T // TCHUNK
NSUB = TCHUNK // 128       # 128-row subtiles per t-tile
DC = D // 128              # contraction chunks
F32 = mybir.dt.float32
F32R = mybir.dt.float32r

# Populated on every kernel() call: list of (name, BassKernelResults)
LAST_RUN = []


def _load_w1_f32(nc, const, dtype):
    """W1 [D, D] -> SBUF [128, DC, D]; slice [:, dc, e0:e1] is lhsT [128d, e]."""
    w1_d = nc.dram_tensor("w1", [D, D], dtype, kind="ExternalInput")
    w1sb = const.tile([128, DC, D], dtype, tag="w1sb")
    nc.sync.dma_start(out=w1sb, in_=w1_d.rearrange("(c p) e -> p c e", p=128))
    return w1sb


def _load_vec(nc, const, name, dtype, tag):
    """[D] vector -> SBUF [128, DC]; column c holds elements c*128..c*128+127."""
    v_d = nc.dram_tensor(name, [D], dtype, kind="ExternalInput")
    vsb = const.tile([128, DC], dtype, tag=tag)
    nc.sync.dma_start(out=vsb, in_=v_d.rearrange("(c p) -> p c", p=128))
    return vsb


def _score_pass(nc, tc, pools, ident, w1sb, b1sb, w2sb, x_ap, s_out_ap,
                n_rows, mm_dtype):
    """Compute s = tanh(x @ W1 + b1) @ w2 for n_rows rows of x.

    x_ap:     DRAM AP [n_rows, D]
    s_out_ap: DRAM AP [1, n_rows]
    Processes in column blocks of TCHUNK rows stacked along the free dim.
    """
    const, xin, xtp, hp, sp, pst, psh, pss = pools

    nchunks = n_rows // TCHUNK
    for i in range(nchunks):
        # ---- load x rows [TCHUNK, D] as 4 subtiles of 128 rows
        xn = xin.tile([128, NSUB, D], F32, tag="xn")
        nc.sync.dma_start(
            out=xn,
            in_=x_ap[i * TCHUNK:(i + 1) * TCHUNK, :].rearrange(
                "(n p) d -> p n d", p=128),
        )
        # ---- transpose to xT [128d, DC, TCHUNK]
        xT = xtp.tile([128, DC, TCHUNK], mm_dtype, tag="xT")
        for dc in range(DC):
            ptr = pst.tile([128, TCHUNK], F32, tag="ptr")
            for n in range(NSUB):
                nc.tensor.matmul(
                    ptr[:, n * 128:(n + 1) * 128],
                    xn[:, n, dc * 128:(dc + 1) * 128],
                    ident,
                    is_transpose=True,
                    start=(n == 0),
                    stop=(n == NSUB - 1),
                )
            nc.any.tensor_copy(xT[:, dc, :], ptr)
        # ---- hT = tanh(W1.T @ xT + b1), then s += w2.T @ hT
        ps_s = pss.tile([1, TCHUNK], F32, tag="ps_s")
        for ec in range(DC):
            ph = psh.tile([128, TCHUNK], F32, tag="ph")
            for dc in range(DC):
                nc.tensor.matmul(
                    ph,
                    w1sb[:, dc, ec * 128:(ec + 1) * 128],
                    xT[:, dc, :],
                    start=(dc == 0),
                    stop=(dc == DC - 1),
                )
            hT = hp.tile([128, TCHUNK], mm_dtype, tag="hT")
            nc.scalar.activation(
                hT, ph, mybir.ActivationFunctionType.Tanh,
                bias=b1sb[:, ec:ec + 1],
            )
            nc.tensor.matmul(
                ps_s, w2sb[:, ec:ec + 1], hT,
                start=(ec == 0), stop=(ec == DC - 1),
            )
        s_sb = sp.tile([1, TCHUNK], F32, tag="s_sb")
        nc.vector.tensor_copy(s_sb, ps_s)
        nc.sync.dma_start(
            out=s_out_ap[:, i * TCHUNK:(i + 1) * TCHUNK], in_=s_sb)


def _mk_pools(tc, ctx_list, psh_bufs=4):
    mk = tc.tile_pool
    pools = (
        ctx_list.enter_context(mk(name="const", bufs=1)),
        ctx_list.enter_context(mk(name="xin", bufs=2)),
        ctx_list.enter_context(mk(name="xtp", bufs=2)),
        ctx_list.enter_context(mk(name="hp", bufs=2)),
        ctx_list.enter_context(mk(name="sp", bufs=2)),
        ctx_list.enter_context(mk(name="pst", bufs=2, space="PSUM")),
        ctx_list.enter_context(mk(name="psh", bufs=psh_bufs, space="PSUM")),
        ctx_list.enter_context(mk(name="pss", bufs=2, space="PSUM")),
    )
    return pools


def _maybe_repeat(tc, ctx, repeat):
    """Wrap everything emitted afterwards in an on-device repeat loop
    (used only for benchmarking: device-time = Δwall / Δrepeat)."""
    if repeat > 1:
        ctx.enter_context(tc.For_i(0, repeat, 1))


def build_k1(repeat=1):
    """Approximate (f32r) scores for the core's BPC batches."""
    from contextlib import ExitStack
    nc = bacc.Bacc()
    x_d = nc.dram_tensor("x", [BPC, T, D], F32, kind="ExternalInput")
    s_d = nc.dram_tensor("s", [BPC, T], F32, kind="ExternalOutput")
    with TileContext(nc) as tc:
        with ExitStack() as ctx:
            pools = _mk_pools(tc, ctx)
            const = pools[0]
            ident = const.tile([128, 128], F32, tag="ident")
            make_identity(nc, ident)
            w1sb = _load_w1_f32(nc, const, F32R)
            b1sb = _load_vec(nc, const, "b1", F32, "b1sb")
            w2sb = _load_vec(nc, const, "w2", F32R, "w2sb")
            _maybe_repeat(tc, ctx, repeat)
            for b in range(BPC):
                _score_pass(
                    nc, tc, pools, ident, w1sb, b1sb, w2sb,
                    x_d[b, :, :], s_d[b:b + 1, :], T, F32R)
    nc.finalize()
    return nc


def build_k2(repeat=1):
    """Exact fp32 scores for the BPC*WIN host-gathered window rows."""
    from contextlib import ExitStack
    nrows = BPC * WIN
    nc = bacc.Bacc()
    xg_d = nc.dram_tensor("xg", [nrows, D], F32, kind="ExternalInput")
    s_d = nc.dram_tensor("sref", [1, nrows], F32, kind="ExternalOutput")
    with TileContext(nc) as tc:
        with ExitStack() as ctx:
            pools = _mk_pools(tc, ctx)
            const = pools[0]
            ident = const.tile([128, 128], F32, tag="ident")
            make_identity(nc, ident)
            w1sb = _load_w1_f32(nc, const, F32)
            b1sb = _load_vec(nc, const, "b1", F32, "b1sb")
            w2sb = _load_vec(nc, const, "w2", F32, "w2sb")
            _maybe_repeat(tc, ctx, repeat)
            _score_pass(
                nc, tc, pools, ident, w1sb, b1sb, w2sb,
                xg_d[:, :], s_d, nrows, F32)
    nc.finalize()
    return nc


def build_k3(repeat=1):
    """Mask + softmax + out = x * w, given merged scores and thresholds."""
    from contextlib import ExitStack
    nc = bacc.Bacc()
    x_d = nc.dram_tensor("x", [BPC, T, D], F32, kind="ExternalInput")
    sf_d = nc.dram_tensor("sf", [BPC, T], F32, kind="ExternalInput")
    thr_d = nc.dram_tensor("thr", [BPC, 1], F32, kind="ExternalInput")
    out_d = nc.dram_tensor("out", [BPC, T, D], F32, kind="ExternalOutput")
    w_d = nc.dram_tensor("w", [BPC, T], F32, kind="ExternalOutput")
    NB = T // 128              # 32 t-blocks per batch
    OCH = 4                    # t-blocks per output DMA chunk
    with TileContext(nc) as tc:
        with ExitStack() as ctx:
            const = ctx.enter_context(tc.tile_pool(name="const", bufs=1))
            sm = ctx.enter_context(tc.tile_pool(name="sm", bufs=2))
            xin = ctx.enter_context(tc.tile_pool(name="xin", bufs=3))
            op = ctx.enter_context(tc.tile_pool(name="op", bufs=3))
            psp = ctx.enter_context(tc.tile_pool(name="psp", bufs=2, space="PSUM"))
            ident32 = const.tile([32, 32], F32, tag="id32")
            make_identity(nc, ident32)
            ones32 = const.tile([32, 32], F32, tag="ones32")
            nc.vector.memset(ones32, 1.0)
            _maybe_repeat(tc, ctx, repeat)
            for b in range(BPC):
                s_np = sm.tile([32, 128], F32, tag="s_np")
                nc.sync.dma_start(
                    out=s_np, in_=sf_d[b, :].rearrange("(n p) -> n p", p=128))
                thr_sb = sm.tile([32, 1], F32, tag="thr_sb")
                nc.sync.dma_start(
                    out=thr_sb, in_=thr_d[b, :].to_broadcast([32, 1]))
                # mask = s > thr ; em = exp(s) * mask
                mask = sm.tile([32, 128], F32, tag="mask")
                nc.vector.tensor_scalar(
                    mask, s_np, thr_sb, None, op0=mybir.AluOpType.is_gt)
                ex = sm.tile([32, 128], F32, tag="ex")
                nc.scalar.activation(ex, s_np, mybir.ActivationFunctionType.Exp)
                em = sm.tile([32, 128], F32, tag="em")
                nc.vector.tensor_tensor(
                    out=em, in0=ex, in1=mask, op=mybir.AluOpType.mult)
                # total = sum(em) replicated across partitions, winv = 1/total
                part = sm.tile([32, 1], F32, tag="part")
                nc.vector.tensor_reduce(
                    part, em, axis=mybir.AxisListType.X, op=mybir.AluOpType.add)
                ps_tot = psp.tile([32, 1], F32, tag="ps_tot")
                nc.tensor.matmul(ps_tot, ones32, part, start=True, stop=True)
                tot = sm.tile([32, 1], F32, tag="tot")
                nc.vector.tensor_copy(tot, ps_tot)
                winv = sm.tile([32, 1], F32, tag="winv")
                nc.vector.reciprocal(winv, tot)
                # weights (row layout) -> DRAM ; transpose for the multiply
                w_np = sm.tile([32, 128], F32, tag="w_np")
                nc.vector.tensor_scalar_mul(w_np, em, winv)
                nc.sync.dma_start(
                    out=w_d[b, :].rearrange("(n p) -> n p", p=128), in_=w_np)
                ps_wt = psp.tile([128, 32], F32, tag="ps_wt")
                nc.tensor.transpose(ps_wt, w_np, ident32)
                w_str = sm.tile([128, 32], F32, tag="w_str")
                nc.vector.tensor_copy(w_str, ps_wt)
                # out = x * w
                for c in range(NB // OCH):
                    xt = xin.tile([128, OCH, D], F32, tag="xt")
                    nc.sync.dma_start(
                        out=xt,
                        in_=x_d[b, c * OCH * 128:(c + 1) * OCH * 128, :]
                        .rearrange("(n p) d -> p n d", p=128),
                    )
                    ot = op.tile([128, OCH, D], F32, tag="ot")
                    for m in range(OCH):
                        nc.vector.tensor_scalar_mul(
                            ot[:, m, :], xt[:, m, :],
                            w_str[:, c * OCH + m:c * OCH + m + 1])
                    nc.sync.dma_start(
                        out=out_d[b, c * OCH * 128:(c + 1) * OCH * 128, :]
                        .rearrange("(n p) d -> p n d", p=128),
                        in_=ot,
                    )
    nc.finalize()
    return nc


_PROGS = {}


def _get_prog(name):
    if name not in _PROGS:
        _PROGS[name] = {"k1": build_k1, "k2": build_k2, "k3": build_k3}[name]()
    return _PROGS[name]


def _run(name, in_maps):
    res = run_bass_kernel_spmd(_get_prog(name), in_maps, core_ids=list(range(NCORES)))
    LAST_RUN.append((name, res))
    return res


def kernel(x, W1, b1, w2, b2):
    LAST_RUN.clear()
    x = np.ascontiguousarray(np.asarray(x, dtype=np.float32))
    W1 = np.ascontiguousarray(np.asarray(W1, dtype=np.float32))
    b1 = np.ascontiguousarray(np.asarray(b1, dtype=np.float32)).reshape(D)
    w2v = np.ascontiguousarray(np.asarray(w2, dtype=np.float32)).reshape(D)
    b2s = float(np.asarray(b2).reshape(-1)[0])

    # ---- K1: approximate scores
    in1 = [
        {"x": x[c * BPC:(c + 1) * BPC], "w1": W1, "b1": b1, "w2": w2v}
        for c in range(NCORES)
    ]
    r1 = _run("k1", in1)
    s1 = np.concatenate([r1.results[c]["s"] for c in range(NCORES)], axis=0)
    s1 = s1 + np.float32(b2s)                       # [B, T]

    # ---- host: rank-window selection
    order = np.argsort(s1, axis=1, kind="stable")
    win_idx = order[:, KEEP - HALF:KEEP + HALF]     # [B, WIN]
    xg = np.ascontiguousarray(
        np.take_along_axis(x, win_idx[:, :, None], axis=1))  # [B, WIN, D]

    # ---- K2: exact fp32 scores for the window rows
    in2 = [
        {"xg": xg[c * BPC:(c + 1) * BPC].reshape(BPC * WIN, D),
         "w1": W1, "b1": b1, "w2": w2v}
        for c in range(NCORES)
    ]
    r2 = _run("k2", in2)
    sref = np.concatenate(
        [r2.results[c]["sref"].reshape(BPC, WIN) for c in range(NCORES)], axis=0)
    sref = sref + np.float32(b2s)                   # [B, WIN]

    # ---- host: merge + exact per-row threshold
    s_final = s1.copy()
    np.put_along_axis(s_final, win_idx, sref, axis=1)
    thr = np.partition(s_final, KEEP - 1, axis=1)[:, KEEP - 1]  # [B]
    counts = (s_final > thr[:, None]).sum(axis=1)
    assert np.all(counts == KEEP), f"threshold tie rows: {np.where(counts != KEEP)}"

    # ---- K3: mask + softmax + multiply
    in3 = [
        {"x": x[c * BPC:(c + 1) * BPC],
         "sf": s_final[c * BPC:(c + 1) * BPC],
         "thr": thr[c * BPC:(c + 1) * BPC].reshape(BPC, 1).astype(np.float32)}
        for c in range(NCORES)
    ]
    r3 = _run("k3", in3)
    out = np.concatenate([r3.results[c]["out"] for c in range(NCORES)], axis=0)
    w_out = np.concatenate([r3.results[c]["w"] for c in range(NCORES)], axis=0)
    return out, w_out
